# revision 69
# baseline (speedup 1.0000x reference)
"""Trainium2 Bass kernel for the dual cross-attention similarity module.

Math (per query q of 300, way w of 5): qkv from shared W; B->A attention
(25 b-tokens over 125 a-tokens/way) and A->B attention (125 a-tokens/way
over 25 b-tokens); outputs are negated squared Frobenius distances between
v and softmax-reconstructions. Sharding: queries split 40/core across 8
cores; features_a / W / consts replicated; no collectives.

Per-core design (v4, from the v3 baseline):
  * Loads ordered wt(512-padded rows) -> fa -> fb -> consts; PE/Act warm
    from memset tiles so nothing waits on the const blob.
  * qkv evacuates into contiguous per-side tiles; the DoubleRow [64,2,..]
    fold layouts are produced by 3 batched sbuf->sbuf DMA pairs (qav, qb,
    kv) instead of 12 singles. vA_tok comes straight from DR matmuls
    (stationary fa, moving W_v) - no ident transposes. sqa/napb hoisted
    off the output tail; sqa squares and vbB copies deferred past the
    first five exps in the Act queue.
  * Phase B chunk-0 scores run fold-free (plain fp8 on the unfolded
    tiles) so they start right after the q_b evacs; chunk-1 uses DR via
    the folds. Per-way stream: score -> exp -> R -> {c1 DVE, c2 Act(w0-2)
    / rpS+Pool(w3-4)} -> ow thin-reductions; nb rides row 5 of the 8-row
    Z group in the zcc bank.
  * Phase C chunks (512,128): per group DR scores + DR cross-gram, aeb =
    blockdiag-gram @ E; egp/eab DVE with every-3rd egp through Act-evac
    (Pool or DVE-2x mul); Z/U/V accumulate into 32-aligned rows of two
    banks; chunk tails stage Z/U/V to SBUF (chunk-0) or read psum direct
    (last chunk) and stream sq out per-chunk. The b-tail (transposes +
    rational math) interleaves between chunks; its indicator matmuls are
    emitted late so the PE queue never parks on Pool.
  * PSUM: 8 banks via cross-phase tag reuse {pq:4,ptp:2 | sba:2, rp:2,
    zcc:2, zuv:1, vp:1}.
Cost-model timeline: 52.1us/core (v3 baseline: 60.6us).
"""

import numpy as np
import ml_dtypes

import concourse.bass as bass
import concourse.bacc as bacc
import concourse.tile as tile
from concourse import mybir
from concourse.bass_utils import run_bass_kernel_spmd

F32 = mybir.dt.float32
BF16 = mybir.dt.bfloat16
FP8 = mybir.dt.float8e4
AL = mybir.AluOpType
AF = mybir.ActivationFunctionType
AX = mybir.AxisListType
DR = mybir.MatmulPerfMode.DoubleRow

WS = 2.0                      # host-side W scale (outputs carry WS^2)
SCALE = 0.08838834764831845 / (WS * WS)   # 1/sqrt(128) / WS^2
N_CORES = 8
NQ = 40          # queries per core
LB = 25          # b tokens per query
LA = 125         # a tokens per way
NW = 5
TB = NQ * LB     # 1000
TAP = NW * 128   # 640 (a tokens padded to 128/way)

# consts blob (bf16) column offsets
C_OW8 = 0         # [125,5,8]  way indicator, 8-wide (col w)
C_OW128 = 40      # [128,5,5]
C_BO41 = 65       # [125,8,41] query block indicator, col 40 zero
C_BDM = 393       # [125,125]
C_ONES = 518      # [128,1]
C_ONES40 = 519    # [1,40]
C_N8 = 559        # [128,8]   ones in col 5 (nb row)
C_N41 = 567       # [128,41]  ones in col 40 (na row)
NCB = 608
# f32 blob offsets
F_IND = 0         # [128,8,40]
F_ID = 320        # [128,128]
NCF = 448


def build_nc():
    nc = bacc.Bacc("TRN2", target_bir_lowering=False, debug=False)

    fa_d = nc.dram_tensor("fa", [640, TAP], FP8, kind="ExternalInput")
    fb_d = nc.dram_tensor("fb", [640, TB], FP8, kind="ExternalInput")
    wt_d = nc.dram_tensor("wt", [640, 512], FP8, kind="ExternalInput")
    cb_d = nc.dram_tensor("cb", [128, NCB], BF16, kind="ExternalInput")
    cf_d = nc.dram_tensor("cf", [128, NCF], F32, kind="ExternalInput")
    sq_d = nc.dram_tensor("sq", [NQ, NW], F32, kind="ExternalOutput")
    qs_d = nc.dram_tensor("qs", [NQ, NW], F32, kind="ExternalOutput")

    with tile.TileContext(nc) as tc:
        with (
            tc.tile_pool(name="const", bufs=1) as const,
            tc.tile_pool(name="feat", bufs=1) as feat,
            tc.tile_pool(name="persist", bufs=1) as persist,
            tc.tile_pool(name="ew", bufs=1) as ew,
            tc.tile_pool(name="work", bufs=2) as work,
        ):
            psB1_cm = tc.tile_pool(name="psB1", bufs=1, space="PSUM")
            psB1 = psB1_cm.__enter__()
            psA_cm = tc.tile_pool(name="psA", bufs=1, space="PSUM")
            psA = psA_cm.__enter__()

            # ---------------- loads (wt -> fa -> fb -> cb -> cf) -----------
            wrm = feat.tile([128, 128], BF16)
            nc.gpsimd.memset(wrm, 0.0)
            wt = feat.tile([128, 6, 512], FP8)
            nc.gpsimd.memset(wt[:, 5, :], 0.0)
            nc.sync.dma_start(out=wt[:, 0:5, :],
                              in_=wt_d.rearrange("(cb c) e -> c cb e", c=128))
            fa = feat.tile([128, 6, TAP], FP8)
            nc.gpsimd.memset(fa[:, 5, :], 0.0)
            nc.sync.dma_start(out=fa[:, 0:5, :],
                              in_=fa_d.rearrange("(cb c) t -> c cb t", c=128))
            fb = feat.tile([128, 6, TB], FP8)
            nc.gpsimd.memset(fb[:, 5, :], 0.0)
            nc.sync.dma_start(out=fb[:, 0:5, :],
                              in_=fb_d.rearrange("(cb c) t -> c cb t", c=128))
            cb = const.tile([128, NCB], BF16)
            nc.sync.dma_start(out=cb, in_=cb_d[:])
            cf = const.tile([128, NCF], F32)
            nc.sync.dma_start(out=cf, in_=cf_d[:])
            ow8 = cb[0:125, C_OW8:C_OW8 + 40].rearrange("p (w c) -> p w c", w=5)
            ow128 = cb[:, C_OW128:C_OW128 + 25].rearrange("p (w c) -> p w c", w=5)
            bo41 = cb[0:125, C_BO41:C_BO41 + 328].rearrange("p (g c) -> p g c", g=8)
            bdm = cb[0:125, C_BDM:C_BDM + 125]
            ones128 = cb[:, C_ONES:C_ONES + 1]
            ones40 = cb[0:1, C_ONES40:C_ONES40 + 40]
            onesn8 = cb[:, C_N8:C_N8 + 8]
            onesn41 = cb[:, C_N41:C_N41 + 41]
            indB = cf[:, F_IND:F_IND + 320].rearrange("p (g c) -> p g c", g=8)
            identf = cf[:, F_ID:F_ID + 128]

            # warm ACT tables + PE p-state ramp (no load dependencies)
            warm = work.tile([1, 8], F32, tag="warm")
            nc.scalar.activation(out=warm[:, 0:1], in_=wrm[0:1, 0:1], func=AF.Exp)
            wdum = psA.tile([128, 512], F32, tag="pq", bufs=4)
            for i in range(10):
                nc.tensor.matmul(wdum[:, 0:128], wrm, wrm, start=True, stop=True)

            # ---------------- persist tiles ----------------
            qkva8 = persist.tile([128, 3, TAP], FP8)
            sqa = persist.tile([128, TAP], BF16)
            qkvb8 = persist.tile([128, 3, 8, 128], FP8)
            nc.gpsimd.memset(qkvb8[:, :, :, 125:128], 0.0)
            vbB = persist.tile([128, TB], BF16)
            vA_tok = persist.tile([125, 5, 128], BF16)
            zs = persist.tile([128, 1024], F32)
            nc.gpsimd.memset(zs[:, TB:1024], 1.0)

            # ---------------- phase A: qkv ----------------
            # qkva8 slots: 0=k, 1=q, 2=v ; qkvb8 slots: 0=q, 1=k, 2=v
            qkvaD = persist.tile([64, 2, 3, TAP], FP8)
            qkvbD = persist.tile([64, 2, 3, 8, 128], FP8)
            kaD = qkvaD[:, :, 0, :]
            qaD = qkvaD[:, :, 1, :]
            vaD = qkvaD[:, :, 2, :]
            qbD = qkvbD[:, :, 0]
            kbD = qkvbD[:, :, 1]
            vbD = qkvbD[:, :, 2]

            def qkv_a_e(e, slot, defer=None):
                done = 0
                for cw in (384, 256):
                    pq = psA.tile([128, 512], F32, tag="pq", bufs=4)
                    for j in range(3):
                        nc.tensor.matmul(
                            pq[:, 0:cw],
                            wt[:, 2 * j:2 * j + 2, e * 128:(e + 1) * 128],
                            fa[:, 2 * j:2 * j + 2, done:done + cw],
                            start=(j == 0), stop=(j == 2), perf_mode=DR)
                    dst = qkva8[:, slot, done:done + cw]
                    if e == 1:
                        nc.scalar.copy(out=dst, in_=pq[:, 0:cw])
                    else:
                        nc.vector.tensor_copy(out=dst, in_=pq[:, 0:cw])
                    if e == 2:
                        defer.append((done, cw, pq))
                    done += cw

            def qkv_b_e(e, slot, defer=None):
                for ci in range(2):
                    c0 = ci * 500
                    pq = psA.tile([128, 512], F32, tag="pq", bufs=4)
                    for j in range(3):
                        nc.tensor.matmul(
                            pq[:, 0:500],
                            wt[:, 2 * j:2 * j + 2, e * 128:(e + 1) * 128],
                            fb[:, 2 * j:2 * j + 2, c0:c0 + 500],
                            start=(j == 0), stop=(j == 2), perf_mode=DR)
                    dst = qkvb8[:, slot, 4 * ci:4 * ci + 4, 0:125]
                    src = pq[:, 0:500].rearrange("p (g t) -> p g t", g=4)
                    if e == 0:
                        nc.scalar.copy(out=dst, in_=src)
                    elif e == 1:
                        nc.vector.tensor_copy(out=dst, in_=src)
                    else:
                        nc.vector.tensor_copy(out=dst, in_=src)
                        defer.append((c0, pq))

            def fold_a(s0, s1):
                nc.sync.dma_start(out=qkvaD[:, 0, s0:s1],
                                  in_=qkva8[0:64, s0:s1])
                nc.sync.dma_start(out=qkvaD[:, 1, s0:s1],
                                  in_=qkva8[64:128, s0:s1])

            def fold_b(s0, s1):
                nc.sync.dma_start(out=qkvbD[:, 0, s0:s1],
                                  in_=qkvb8[0:64, s0:s1])
                nc.sync.dma_start(out=qkvbD[:, 1, s0:s1],
                                  in_=qkvb8[64:128, s0:s1])

            qkv_a_e(1, 0)          # k_a (Act evacs)
            fold_a(0, 1)           # kaD (for chunk-1 DR scores)
            qkv_a_e(0, 1)          # q_a (DVE evacs)
            qkv_b_e(0, 0)          # q_b (Act evacs, ahead of everything else)
            fold_b(0, 1)           # qbD (only needed for chunk-1 DR scores)
            sqa_defer = []
            qkv_a_e(2, 2, sqa_defer)  # v_a (sqa squares deferred)
            fold_a(1, 3)           # qaD, vaD
            # vA_tok directly via DR matmuls: (W_v x_a)^T per way
            ptp0 = psA.tile([128, 4, 128], F32, tag="ptp", bufs=2,
                            padded_shape=[128, 4, 128])
            ptp1 = psA.tile([128, 128], F32, tag="ptp", bufs=2,
                            padded_shape=[128, 512])
            for w in range(5):
                dstp = ptp0[:, w, :] if w < 4 else ptp1
                for j in range(3):
                    nc.tensor.matmul(
                        dstp,
                        fa[:, 2 * j:2 * j + 2, w * 128:(w + 1) * 128],
                        wt[:, 2 * j:2 * j + 2, 256:384],
                        start=(j == 0), stop=(j == 2), perf_mode=DR)
            nc.vector.tensor_copy(out=vA_tok[:, 0:4, :], in_=ptp0[0:125])
            nc.vector.tensor_copy(out=vA_tok[:, 4, :], in_=ptp1[0:125])

            # B chunk-0 scores fold-free (plain fp8 from the unfolded tiles):
            # they only need the k_a / q_b evacuations, not the fold DMAs.
            b0_scores = []
            for w in range(5):
                sba = psB1.tile([128, 512], F32, tag="sba", bufs=2)
                nc.tensor.matmul(
                    sba[:, 0:500],
                    qkva8[:, 0, w * 128:(w + 1) * 128],
                    qkvb8[:, 0, 0:4, 0:125],
                    start=True, stop=True)
                e_w = ew.tile([128, 500], BF16, tag="ew", bufs=5)
                nc.scalar.activation(out=e_w, in_=sba[:, 0:500],
                                     func=AF.Exp, scale=SCALE)
                b0_scores.append(e_w)
                if w == 1:
                    qkv_b_e(1, 1)          # k_b (DVE evacs)
                    vb_defer = []
                    qkv_b_e(2, 2, vb_defer)  # v_b (vbB deferred past exps)
                    fold_b(1, 3)           # kbD + vbD together
            for done, cw, pq in sqa_defer:
                nc.scalar.activation(out=sqa[:, done:done + cw],
                                     in_=pq[:, 0:cw], func=AF.Square)
            for c0, pq in vb_defer:
                nc.scalar.copy(out=vbB[:, c0:c0 + 500], in_=pq[:, 0:500])
            sqb = work.tile([128, TB], BF16, tag="sqb")
            nc.gpsimd.tensor_mul(sqb, vbB, vbB)
            # napb = sum_w' n_a[w'] per query (same for all ways): only needs
            # sqa, so hoist the whole chain off the critical tail.
            pna = psA.tile([128, 512], F32, tag="pq", bufs=4)
            nc.tensor.matmul(pna[0:1, 0:512], ones128, sqa[:, 0:512],
                             start=True, stop=True)
            pna2 = psA.tile([128, 512], F32, tag="pq", bufs=4)
            nc.tensor.matmul(pna2[0:1, 0:128], ones128, sqa[:, 512:640],
                             start=True, stop=True)
            naF = work.tile([1, 5], F32, tag="naF")
            nc.vector.tensor_reduce(
                out=naF[:, 0:4],
                in_=pna[0:1, 0:512].rearrange("p (w l) -> p w l", w=4),
                op=AL.add, axis=AX.X)
            nc.vector.tensor_reduce(
                out=naF[:, 4:5],
                in_=pna2[0:1, 0:128].rearrange("p (w l) -> p w l", w=1),
                op=AL.add, axis=AX.X)
            naB = work.tile([1, 5], BF16, tag="naB")
            nc.scalar.copy(out=naB, in_=naF)
            pnapb = psA.tile([128, 512], F32, tag="pq", bufs=4)
            nc.tensor.matmul(pnapb[0:40, 0:5], ones40, naB,
                             start=True, stop=True)
            napbS = work.tile([40, 5], F32, tag="napbS")
            nc.scalar.mul(out=napbS, in_=pnapb[0:40, 0:5],
                          mul=1.0 / (WS * WS))
            psA_cm.__exit__(None, None, None)
            psB2_cm = tc.tile_pool(name="psB2", bufs=1, space="PSUM")
            psB2 = psB2_cm.__enter__()
            psC_cm = tc.tile_pool(name="psC", bufs=1, space="PSUM")
            psC = psC_cm.__enter__()

            # blockdiag grams are produced 2 groups ahead of use
            bds = persist.tile([125, 8, 125], BF16)
            c_pre = {}

            def emit_c_score(ci, cn, c0, g):
                sab = psB1.tile([128, 512], F32, tag="sba", bufs=2)
                nc.tensor.matmul(
                    sab[:, 0:cn], kbD[:, :, g, :],
                    qaD[:, :, c0:c0 + cn],
                    start=True, stop=True, perf_mode=DR)
                eg2 = ew.tile([128, 512], BF16, tag="eg2", bufs=3)
                nc.scalar.activation(out=eg2[:, 0:cn], in_=sab[:, 0:cn],
                                     func=AF.Exp, scale=SCALE)
                c_pre[(ci, g)] = eg2

            # ---------------- phase B: B attends A ----------------
            # zcc bank rows: Z 0:5, c1 32:37, c2 64:69, nb 96
            for ci in range(2):
                c0 = ci * 500
                if ci == 1 and False:
                    emit_c_score(0, 384, 0, 0)
                    emit_c_score(0, 384, 0, 1)
                zcc = psB2.tile([128, 512], F32, tag="zcc", bufs=2)
                pend = None
                for w in range(5):
                    if ci == 0:
                        e_w = b0_scores[w]
                    else:
                        sba = psB1.tile([128, 512], F32, tag="sba", bufs=2)
                        nc.tensor.matmul(
                            sba[:, 0:500],
                            kaD[:, :, w * 128:(w + 1) * 128],
                            qbD[:, :, 4 * ci:4 * ci + 4, 0:125],
                            start=True, stop=True, perf_mode=DR)
                        e_w = ew.tile([128, 500], BF16, tag="ew", bufs=5)
                        nc.scalar.activation(out=e_w, in_=sba[:, 0:500],
                                             func=AF.Exp, scale=SCALE)
                    rp = psB2.tile([128, 512], F32, tag="rp", bufs=2)
                    nc.tensor.matmul(rp[:, 0:500], vA_tok[:, w, :],
                                     e_w[0:125, :], start=True, stop=True)
                    nc.tensor.matmul(zcc[0:8, 0:500], ow8[:, w, :],
                                     e_w[0:125, :],
                                     start=(w == 0), stop=False)
                    if w == 4:
                        # nb row rides as row 5 of the Z accumulation group
                        nc.tensor.matmul(zcc[0:8, 0:500], onesn8,
                                         sqb[:, c0:c0 + 500],
                                         start=False, stop=True)
                    c1sb = work.tile([128, 500], BF16, tag="c1sb", bufs=3)
                    c2sb = work.tile([128, 500], BF16, tag="c2sb", bufs=3)
                    if w >= 3:
                        rpS = work.tile([128, 500], BF16, tag="rpS", bufs=3)
                        nc.vector.tensor_copy(out=rpS, in_=rp[:, 0:500])
                        nc.gpsimd.tensor_mul(c2sb, rpS, rpS)
                        nc.vector.tensor_mul(c1sb, rp[:, 0:500],
                                             vbB[:, c0:c0 + 500])
                    else:
                        nc.vector.tensor_mul(c1sb, rp[:, 0:500],
                                             vbB[:, c0:c0 + 500])
                        nc.scalar.activation(out=c2sb, in_=rp[:, 0:500],
                                             func=AF.Square)
                    if pend is not None:
                        pw, pc1, pc2 = pend
                        nc.tensor.matmul(zcc[32:37, 0:500], ow128[:, pw, :],
                                         pc1, start=(pw == 0), stop=False)
                        nc.tensor.matmul(zcc[64:69, 0:500], ow128[:, pw, :],
                                         pc2, start=(pw == 0), stop=False)
                    pend = (w, c1sb, c2sb)
                pw, pc1, pc2 = pend
                nc.tensor.matmul(zcc[32:37, 0:500], ow128[:, pw, :], pc1,
                                 start=False, stop=True)
                nc.tensor.matmul(zcc[64:69, 0:500], ow128[:, pw, :], pc2,
                                 start=False, stop=True)
                if ci == 0:
                    nc.scalar.copy(out=zs[:, c0:c0 + 500], in_=zcc[:, 0:500])
                else:
                    nc.vector.tensor_copy(out=zs[:, c0:c0 + 500],
                                          in_=zcc[:, 0:500])

            # ---------------- phase C (+ B tail interleaved) ----------------
            b_tail_state = {}

            def emit_b_tail():
                # zs -> [128 l, 8 slice, 70] transposed slots
                TtS = persist.tile([128, 8, 70], F32)
                for half in range(2):
                    Tt = psB1.tile([128, 4, 128], F32, tag="sba", bufs=2,
                                   padded_shape=[128, 4, 128])
                    for sl in range(4):
                        s = half * 4 + sl
                        nc.tensor.transpose(Tt[:, sl, 0:69],
                                            zs[0:69, s * 128:(s + 1) * 128],
                                            identf[0:69, 0:69])
                    nc.scalar.copy(out=TtS[:, half * 4:half * 4 + 4, 0:69],
                                   in_=Tt[:, :, 0:69])
                # nb (zcc row 5) into slot 69 for the indicator matmul
                nc.vector.tensor_copy(out=TtS[:, :, 69:70],
                                      in_=TtS[:, :, 5:6])
                rT = work.tile([128, 8, 5], F32, tag="rT")
                nc.vector.reciprocal(out=rT, in_=TtS[:, :, 0:5])
                u1T = work.tile([128, 8, 5], F32, tag="u1T")
                nc.gpsimd.tensor_mul(u1T, TtS[:, :, 64:69], rT)
                t3 = work.tile([128, 8, 5], F32, tag="t3")
                nc.vector.scalar_tensor_tensor(out=t3, in0=TtS[:, :, 32:37],
                                               scalar=-2.0, in1=u1T,
                                               op0=AL.mult, op1=AL.add)
                nc.gpsimd.tensor_mul(TtS[:, :, 64:69], t3, rT)
                b_tail_state["TtS"] = TtS

            def emit_b_tail_out():
                # emitted late so the PE queue never parks on the Pool math
                TtS = b_tail_state["TtS"]
                qsp_t = psB1.tile([128, 512], F32, tag="sba", bufs=2)
                qsp = qsp_t[0:40, 0:6]
                for s in range(8):
                    nc.tensor.matmul(qsp, indB[:, s, :], TtS[:, s, 64:70],
                                     start=(s == 0), stop=(s == 7))
                qsb = work.tile([40, 6], F32, tag="qsb")
                nc.vector.tensor_copy(out=qsb, in_=qsp)
                qs_sb = work.tile([40, 5], F32, tag="qs_sb")
                nc.vector.tensor_scalar(
                    out=qs_sb, in0=qsb[:, 0:5], scalar1=qsb[:, 5:6],
                    scalar2=-1.0 / (WS * WS), op0=AL.add, op1=AL.mult)
                nc.sync.dma_start(out=qs_d[:], in_=qs_sb)


            def emit_bdg(g):
                pgram = psB2.tile([128, 512], F32, tag="rp", bufs=2)
                nc.tensor.matmul(pgram[0:128, 0:128], vbD[:, :, g, :],
                                 vbD[:, :, g, :], start=True, stop=True,
                                 perf_mode=DR)
                nc.vector.tensor_mul(bds[:, g, :], pgram[0:125, 0:125], bdm)

            sq_parts = work.tile([40, 5], F32, tag="sq_parts")
            mul_ctr = [0]
            for ci, (c0, cn, co) in enumerate(((0, 512, 0), (512, 128, 4))):
                nwc = cn // 128
                # zuv rows: Z 0:40, U 64:104 ; vp rows: V 0:40, na 96
                zuv = psC.tile([128, 512], F32, tag="zuv", bufs=1)
                vp = psC.tile([128, 512], F32, tag="vp", bufs=1)
                pendC = None
                for g in range(8):
                    if ci == 0 and g == 0:
                        emit_bdg(0)
                        emit_bdg(1)
                    if (ci, g) in c_pre:
                        eg2 = c_pre[(ci, g)]
                    else:
                        sab = psB1.tile([128, 512], F32, tag="sba", bufs=2)
                        nc.tensor.matmul(
                            sab[:, 0:cn], kbD[:, :, g, :],
                            qaD[:, :, c0:c0 + cn],
                            start=True, stop=True, perf_mode=DR)
                        eg2 = ew.tile([128, 512], BF16, tag="eg2", bufs=3)
                        nc.scalar.activation(out=eg2[:, 0:cn], in_=sab[:, 0:cn],
                                             func=AF.Exp, scale=SCALE)
                    gab = psB2.tile([128, 512], F32, tag="rp", bufs=2)
                    nc.tensor.matmul(
                        gab[:, 0:cn], vbD[:, :, g, :],
                        vaD[:, :, c0:c0 + cn],
                        start=True, stop=True, perf_mode=DR)
                    if ci == 0 and g < 6:
                        emit_bdg(g + 2)
                    e_g = eg2[0:125, 0:cn]
                    nc.tensor.matmul(zuv[0:41, 0:cn], bo41[:, g, :], e_g,
                                     start=(g == 0), stop=(g == 7))
                    egp = work.tile([125, 512], BF16, tag="egp", bufs=3)
                    mul_ctr[0] += 1
                    if mul_ctr[0] % 3 == 0:
                        # shed DVE: evac via Act, multiply on Pool
                        gabS = work.tile([125, 512], BF16, tag="gabS", bufs=3)
                        nc.scalar.copy(out=gabS[:, 0:cn], in_=gab[0:125, 0:cn])
                        nc.gpsimd.tensor_mul(egp[:, 0:cn], e_g, gabS[:, 0:cn])
                    elif mul_ctr[0] % 3 == 1:
                        # Act evac, then 2x-mode bf16 mul on DVE
                        gabS = work.tile([125, 512], BF16, tag="gabS", bufs=3)
                        nc.scalar.copy(out=gabS[:, 0:cn], in_=gab[0:125, 0:cn])
                        nc.vector.tensor_mul(egp[:, 0:cn], e_g, gabS[:, 0:cn])
                    else:
                        nc.vector.tensor_mul(egp[:, 0:cn], e_g,
                                             gab[0:125, 0:cn])
                    aeb = psB2.tile([128, 512], F32, tag="zcc", bufs=2)
                    nc.tensor.matmul(aeb[0:125, 0:cn], bds[:, g, :], e_g,
                                     start=True, stop=True)
                    eab = work.tile([125, 512], BF16, tag="eab", bufs=3)
                    nc.vector.tensor_mul(eab[:, 0:cn], e_g, aeb[0:125, 0:cn])
                    if pendC is not None:
                        pg, pegp, peab = pendC
                        nc.tensor.matmul(zuv[64:105, 0:cn], bo41[:, pg, :],
                                         pegp[:, 0:cn],
                                         start=(pg == 0), stop=False)
                        nc.tensor.matmul(vp[0:41, 0:cn], bo41[:, pg, :],
                                         peab[:, 0:cn],
                                         start=(pg == 0), stop=False)
                    pendC = (g, egp, eab)
                pg, pegp, peab = pendC
                nc.tensor.matmul(zuv[64:105, 0:cn], bo41[:, pg, :],
                                 pegp[:, 0:cn], start=False, stop=True)
                nc.tensor.matmul(vp[0:41, 0:cn], bo41[:, pg, :],
                                 peab[:, 0:cn], start=False, stop=True)
                if ci == 0:
                    # stage Z/U/V to SBUF so zuv/vp free early for chunk 1
                    zS = work.tile([40, 512], F32, tag="zS", bufs=2)
                    nc.scalar.copy(out=zS[:, 0:cn], in_=zuv[0:40, 0:cn])
                    uS = work.tile([40, 512], F32, tag="uS", bufs=2)
                    nc.scalar.copy(out=uS[:, 0:cn], in_=zuv[64:104, 0:cn])
                    vpS = work.tile([40, 512], F32, tag="vpS", bufs=2)
                    nc.vector.tensor_copy(out=vpS[:, 0:cn],
                                          in_=vp[0:40, 0:cn])
                    zin, uin, vin = zS[:, 0:cn], uS[:, 0:cn], vpS[:, 0:cn]
                else:
                    # last chunk: skip staging, read psum directly (latency)
                    zin, uin, vin = (zuv[0:40, 0:cn], zuv[64:104, 0:cn],
                                     vp[0:40, 0:cn])
                rab = work.tile([40, 512], F32, tag="rab")
                nc.vector.reciprocal(out=rab[:, 0:cn], in_=zin)
                u1 = work.tile([40, 512], F32, tag="u1")
                if ci == 0:
                    nc.gpsimd.tensor_mul(u1[:, 0:cn], vin, rab[:, 0:cn])
                else:
                    nc.vector.tensor_mul(u1[:, 0:cn], vin, rab[:, 0:cn])
                u2 = work.tile([40, 512], F32, tag="u2")
                nc.vector.scalar_tensor_tensor(
                    out=u2[:, 0:cn], in0=uin, scalar=2.0,
                    in1=u1[:, 0:cn], op0=AL.mult, op1=AL.subtract)
                if ci == 0:
                    f2 = work.tile([40, 512], F32, tag="f2")
                    nc.gpsimd.tensor_mul(f2[:, 0:cn], u2[:, 0:cn],
                                         rab[:, 0:cn])
                    nc.vector.tensor_reduce(
                        out=sq_parts[:, 0:nwc],
                        in_=f2[:, 0:cn].rearrange(
                            "p (w l) -> p w l", w=nwc)[:, :, 0:125],
                        op=AL.add, axis=AX.X)
                else:
                    f2 = work.tile([40, 512], F32, tag="f2")
                    nc.vector.tensor_mul(f2[:, 0:cn], u2[:, 0:cn],
                                         rab[:, 0:cn])
                    nc.vector.tensor_reduce(
                        out=sq_parts[:, co:co + nwc],
                        in_=f2[:, 0:cn].rearrange(
                            "p (w l) -> p w l", w=nwc)[:, :, 0:125],
                        op=AL.add, axis=AX.X)
                sq_sb = work.tile([40, 5], F32, tag="sq_sb", bufs=2)
                nc.vector.scalar_tensor_tensor(
                    out=sq_sb[:, 0:nwc],
                    in0=sq_parts[:, co:co + nwc],
                    scalar=1.0 / (WS * WS),
                    in1=napbS[:, co:co + nwc],
                    op0=AL.mult, op1=AL.subtract)
                nc.sync.dma_start(out=sq_d[:, co:co + nwc],
                                  in_=sq_sb[:, 0:nwc])
                if ci == 0:
                    emit_b_tail()
                    emit_b_tail_out()
            psC_cm.__exit__(None, None, None)
            psB2_cm.__exit__(None, None, None)
            psB1_cm.__exit__(None, None, None)

    nc.compile()
    return nc


_CACHE = {}


def _get_nc():
    if "nc" not in _CACHE:
        _CACHE["nc"] = build_nc()
    return _CACHE["nc"]


def _consts():
    cb = np.zeros((128, NCB), np.float32)
    ow8 = np.zeros((125, 5, 8), np.float32)
    ow128 = np.zeros((128, 5, 5), np.float32)
    for w in range(5):
        ow8[:, w, w] = 1.0
        ow128[:, w, w] = 1.0
    cb[0:125, C_OW8:C_OW8 + 40] = ow8.reshape(125, 40)
    cb[:, C_OW128:C_OW128 + 25] = ow128.reshape(128, 25)
    bo125 = np.kron(np.eye(5, dtype=np.float32), np.ones((25, 1), np.float32))
    bo41 = np.zeros((125, 8, 41), np.float32)
    for g in range(8):
        bo41[:, g, 5 * g:5 * g + 5] = bo125
    cb[0:125, C_BO41:C_BO41 + 328] = bo41.reshape(125, 328)
    cb[0:125, C_BDM:C_BDM + 125] = np.kron(
        np.eye(5, dtype=np.float32), np.ones((25, 25), np.float32))
    cb[:, C_ONES] = 1.0
    cb[0, C_ONES40:C_ONES40 + 40] = 1.0
    cb[:, C_N8 + 5] = 1.0
    cb[:, C_N41 + 40] = 1.0

    cf = np.zeros((128, NCF), np.float32)
    indB = np.zeros((128, 8, 40), np.float32)
    for s in range(8):
        for p in range(128):
            l = s * 128 + p
            if l < TB:
                indB[p, s, l // 25] = 1.0
    cf[:, F_IND:F_IND + 320] = indB.reshape(128, 320)
    cf[:, F_ID:F_ID + 128] = np.eye(128)
    return (cb.astype(ml_dtypes.bfloat16), cf)


def kernel(features_a, features_b, W):
    features_a = np.asarray(features_a, np.float32)
    features_b = np.asarray(features_b, np.float32)
    W = np.asarray(W, np.float32)
    f8 = ml_dtypes.float8_e4m3

    nq_total = features_b.shape[0]
    fbp = np.zeros((N_CORES * NQ, 640, LB), np.float32)
    fbp[:nq_total] = features_b
    fb_t = np.ascontiguousarray(fbp.transpose(1, 0, 2)).astype(f8)
    fa_pad = np.zeros((640, NW, 128), np.float32)
    fa_pad[:, :, :LA] = features_a.transpose(1, 0, 2)
    fa_t = np.ascontiguousarray(fa_pad.reshape(640, TAP)).astype(f8)
    wt = np.zeros((640, 512), np.float32)
    wt[:, 0:384] = W.T * WS
    wt = np.ascontiguousarray(wt).astype(f8)
    cb, cf = _consts()

    in_maps = []
    for c in range(N_CORES):
        m = {
            "fa": fa_t,
            "fb": np.ascontiguousarray(
                fb_t[:, c * NQ:(c + 1) * NQ, :]).reshape(640, TB),
            "wt": wt,
            "cb": cb,
            "cf": cf,
        }
        in_maps.append(m)

    nc = _get_nc()
    res = run_bass_kernel_spmd(nc, in_maps, core_ids=list(range(N_CORES)))

    sq = np.zeros((N_CORES * NQ, NW), np.float32)
    qs = np.zeros((N_CORES * NQ, NW), np.float32)
    for c in range(N_CORES):
        sq[c * NQ:(c + 1) * NQ] = res.results[c]["sq"]
        qs[c * NQ:(c + 1) * NQ] = res.results[c]["qs"]
    return sq[:nq_total], qs[:nq_total]
